# revision 44
# baseline (speedup 1.0000x reference)
"""Self-contained TRN2 Bass kernel for nn_EuclideanSimilarity.

Full-input contract: kernel(x, W, b) with
  x [4, 4096, 128] f32, W [128, 128] f32, b [128] f32
returns out [4, 4096, 4096] f32 = exp(-pairwise_euclidean_dist(x @ W.T + b)).

Sharding: 8 cores, core c -> (batch c//2, query-half c%2); each core computes
its [2048, 4096] block of the pairwise-similarity matrix against the full
key set of its batch (SPMD — identical program, different input slices).

Per-core pipeline: transpose x via PE, hT = W @ xT + b (fp32r matmul),
then d2[m,n] = sq[m] + sq[n] - 2 h_m.h_n assembled in PSUM. The PE's
fast fp32r path only carries ~12 mantissa bits, which would destroy the
near-diagonal cancellation, so the gram term uses hi/lo split-precision
(h = hi + lo, each fp32r): gram = g_hi.k_hi + g_hi.k_lo + g_lo.k_hi
(with g = -2h), the key norms enter via an augmented K=2 matmul with
fp32r hi/lo rows, and the query norm is added at full fp32 by the DVE
drain (tensor_scalar: out = max(psum + sq_q[m], 0), which also fuses the
relu while evacuating PSUM). sqrt and exp(-x) run on the scalar engine,
explicitly order-chained in batches so the sqrt/exp activation-table
sets are not thrashed, and each 128-row output tile leaves through one
2 MiB contiguous DMA. PSUM cycles through 4x[128,1024] slots, each
completed by 8 back-to-back matmuls, to keep the PE clock-gate warm.
"""

from contextlib import ExitStack

import numpy as np

import concourse.mybir as mybir
import concourse.tile as tile
from concourse.tile import add_dep_helper
from concourse import bacc
from concourse.bass import ts
from concourse.masks import make_identity

F32 = mybir.dt.float32
F32R = mybir.dt.float32r
AF = mybir.ActivationFunctionType
ALU = mybir.AluOpType

B = 4
N = 4096
NQ = 2048  # query rows per core
NK = 4096  # key rows per core
D = 128
TEMPERATURE = 1.0
NQT = NQ // 128  # query tiles per core
N_CORES = 8


def kernel_body(ctx: ExitStack, tc: tile.TileContext, out, xq, xk, W, b):
    nc = tc.nc

    consts = ctx.enter_context(tc.tile_pool(name="consts", bufs=1))
    # first ACT op is a dummy sqrt: loads the sqrt table set up front, so the
    # setup Identity ops (present in every set) keep it resident and the first
    # real sqrt pays no table load at the setup/main boundary
    scrap = consts.tile([1, 8], F32)
    nc.gpsimd.memset(scrap[:], 1.0)
    nc.scalar.activation(scrap[:], scrap[:], AF.Sqrt)
    ident = consts.tile([128, 128], F32)
    make_identity(nc, ident[:])

    w_sb = consts.tile([128, 128], F32)
    nc.sync.dma_start(w_sb[:], W[:, :])
    b_sb = consts.tile([128, 1], F32)
    nc.sync.dma_start(b_sb[:], b[:, :])
    bm2_sb = consts.tile([128, 1], F32)
    nc.scalar.mul(bm2_sb[:], b_sb[:], -2.0)
    ones_f32 = consts.tile([128, 512], F32)
    nc.gpsimd.memset(ones_f32[:], 1.0)
    ones_col = consts.tile([128, 1], F32)    # lhsT for the f32 sq matmul
    nc.vector.tensor_copy(ones_col[:], ones_f32[:, 0:1])

    # persistent main-loop operands (hi/lo split for fp32-grade gram)
    h_pool = ctx.enter_context(tc.tile_pool(name="h", bufs=1))
    hk_hi = h_pool.tile([128, NK], F32R)
    hk_lo = h_pool.tile([128, NK], F32R)
    gq_hi = h_pool.tile([128, NQ], F32R)   # g = -2*h (queries)
    gq_lo = h_pool.tile([128, NQ], F32R)

    aug_pool = ctx.enter_context(tc.tile_pool(name="aug", bufs=1))
    # d2 += sum_k ones2[k,m] * aug_k[k,n] = sq_k_hi[n] + sq_k_lo[n];
    # sq_q[m] is added per-partition by the DVE relu (full fp32, no split)
    aug_k = aug_pool.tile([2, NK], F32R)   # rows: sq_k_hi, sq_k_lo
    ones2 = aug_pool.tile([2, 128], F32R)  # constant lhsT for the aug matmul
    nc.vector.tensor_copy(ones2[:], ones_f32[0:2, 0:128])
    sqq_cols = aug_pool.tile([128, NQT], F32)  # sq_q in column-per-qtile form

    xk_r = xk.rearrange("(t p) d -> p t d", p=128)
    xq_r = xq.rearrange("(t p) d -> p t d", p=128)

    # ---------------- setup phase (scoped pools) ----------------
    with tc.tile_pool(name="setup_sb", bufs=6) as ssb, \
         tc.tile_pool(name="setup_ps", bufs=2, space="PSUM") as sps, \
         tc.tile_pool(name="rows", bufs=1) as rows_pool:

        wt_ps = sps.tile([128, 512], F32, tag="wt", bufs=1)
        nc.tensor.transpose(wt_ps[:, 0:128], w_sb[:], ident[:])
        wt_sb = consts.tile([128, 128], F32R)
        nc.vector.tensor_copy(wt_sb[:], wt_ps[:, 0:128])

        # single-partition staging row for raw query norms (fp32, 4*|h|^2)
        sqq_row = rows_pool.tile([1, NQ], F32)

        def do_chunks(nchunks, x_r, hi_dst, lo_dst, is_q):
            for c in range(nchunks):
                tagn = "q" if is_q else "k"
                xin = ssb.tile([128, 512], F32, tag="xin", name=f"xin_{tagn}{c}")
                nc.sync.dma_start(
                    xin[:].rearrange("p (t d) -> p t d", d=D),
                    x_r[:, 4 * c:4 * c + 4, :],
                )
                tp = sps.tile([128, 512], F32, tag="tp", bufs=3, name=f"tp_{tagn}{c}")
                for j in range(4):
                    nc.tensor.transpose(
                        tp[:, ts(j, 128)], xin[:, ts(j, 128)], ident[:]
                    )
                xt = ssb.tile([128, 512], F32R, tag="xt", name=f"xt_{tagn}{c}")
                nc.scalar.activation(xt[:], tp[:], AF.Identity)
                hps = sps.tile([128, 512], F32, tag="hps", bufs=2, name=f"hps_{tagn}{c}")
                nc.tensor.matmul(hps[:], wt_sb[:], xt[:], start=True, stop=True)
                hf = ssb.tile([128, 512], F32, tag="hf", name=f"hf_{tagn}{c}")
                if is_q:  # g = -2*(W@xT) - 2b
                    nc.scalar.activation(
                        hf[:], hps[:], AF.Identity, bias=bm2_sb[:, 0:1],
                        scale=-2.0,
                    )
                else:
                    nc.scalar.activation(
                        hf[:], hps[:], AF.Identity, bias=b_sb[:, 0:1]
                    )
                # hi/lo split of h (or g)
                nc.gpsimd.tensor_copy(hi_dst[:, ts(c, 512)], hf[:])
                nc.vector.tensor_tensor(
                    lo_dst[:, ts(c, 512)], hf[:], hi_dst[:, ts(c, 512)],
                    ALU.subtract,
                )
                # squared norms, also hi/lo so the K=128 sum keeps f32 grade
                s2f = ssb.tile([128, 512], F32, tag="s2f", name=f"s2f_{tagn}{c}")
                nc.vector.tensor_mul(s2f[:], hf[:], hf[:])
                sqps = sps.tile([128, 512], F32, tag="sqps", bufs=2, name=f"sqps_{tagn}{c}")
                # plain-f32 matmul (2-pass internally) keeps the norm exact
                nc.tensor.matmul(
                    sqps[0:1, :], ones_col[:], s2f[:], start=True, stop=True
                )
                if is_q:  # raw 4*|h|^2; the 1/4 scale is applied at transpose
                    nc.scalar.activation(
                        sqq_row[0:1, ts(c, 512)], sqps[0:1, :], AF.Identity
                    )
                else:
                    nc.scalar.activation(
                        aug_k[0:1, ts(c, 512)], sqps[0:1, :], AF.Identity
                    )
                    sk = rows_pool.tile([1, 512], F32R, tag="sklo", bufs=2,
                                        name=f"sklo{c}")
                    nc.vector.tensor_tensor(
                        sk[:], sqps[0:1, :], aug_k[0:1, ts(c, 512)],
                        ALU.subtract,
                    )
                    nc.sync.dma_start(aug_k[1:2, ts(c, 512)], sk[:])

        do_chunks(NQ // 512, xq_r, gq_hi, gq_lo, True)
        # transpose sq_q row into column-per-qtile layout via tiny PE transposes
        sqq_ps = sps.tile([128, 512], F32, tag="sqps", bufs=2, name="sqq_ps")
        for qt in range(NQT):
            nc.tensor.transpose(
                sqq_ps[:, qt:qt + 1], sqq_row[0:1, ts(qt, 128)], ident[0:1, 0:1]
            )
        nc.vector.tensor_scalar_mul(sqq_cols[:], sqq_ps[:, 0:NQT], 0.25)
        do_chunks(NK // 512, xk_r, hk_hi, hk_lo, False)

    # ---------------- main loop ----------------
    stage_pool = ctx.enter_context(tc.tile_pool(name="stage", bufs=8))
    d2_ps = ctx.enter_context(tc.tile_pool(name="d2", bufs=4, space="PSUM"))
    NC = NK // 512  # 8 key chunks

    last_act = [None]

    def chained_act(*args, chain=True, **kwargs):
        bi = nc.scalar.activation(*args, **kwargs)
        if chain and last_act[0] is not None:
            # arg order: (waiter, dependency) - this op waits on the previous
            add_dep_helper(bi.ins, last_act[0].ins, sync=False,
                           reason="act-table-order")
        last_act[0] = bi
        return bi

    NH = NK // 1024  # 4 slots of 2 key-chunks each
    spans = [(0, 3), (3, 6), (6, 9), (9, 12), (12, 14), (14, 16)]
    assert spans[-1][1] == NQT
    for g0, g1 in spans:
        group = []
        for qt in range(g0, g1):
            st = stage_pool.tile([128, NK], F32, tag="st", name=f"st{qt}")
            for h in range(NH):
                ps = d2_ps.tile([128, 1024], F32, tag="d2", name=f"d2_{qt}_{h}")
                # chunk-major: finish each 512-column before moving on, so the
                # slot completes after 8 back-to-back matmuls
                for cc in range(2):
                    c = 2 * h + cc
                    nc.tensor.matmul(
                        ps[:, ts(cc, 512)], gq_hi[:, ts(qt, 128)],
                        hk_hi[:, ts(c, 512)], start=True, stop=False,
                    )
                    nc.tensor.matmul(
                        ps[:, ts(cc, 512)], gq_hi[:, ts(qt, 128)],
                        hk_lo[:, ts(c, 512)], start=False, stop=False,
                    )
                    nc.tensor.matmul(
                        ps[:, ts(cc, 512)], gq_lo[:, ts(qt, 128)],
                        hk_hi[:, ts(c, 512)], start=False, stop=False,
                    )
                    nc.tensor.matmul(
                        ps[:, ts(cc, 512)], ones2[:],
                        aug_k[:, ts(c, 512)], start=False, stop=True,
                    )
                nc.vector.tensor_scalar(
                    st[:, ts(h, 1024)], ps[:], sqq_cols[:, qt:qt + 1], 0.0,
                    ALU.add, ALU.max,
                )
            if g0 == 0 and qt < 3:
                # pipeline-fill phase: sqrt per 2048-half starts ~2 slots earlier
                chained_act(st[:, 0:2048], st[:, 0:2048], AF.Sqrt)
                chained_act(st[:, 2048:4096], st[:, 2048:4096], AF.Sqrt)
            else:
                chained_act(st[:], st[:], AF.Sqrt)
            group.append((qt, st))
        for qt, st in group:
            if qt == NQT - 1:
                # final tile: halve exp+DMA so the last DMA overlaps the exp
                chained_act(st[:, 0:2048], st[:, 0:2048], AF.Exp,
                            scale=-TEMPERATURE)
                nc.sync.dma_start(out[ts(qt, 128), 0:2048], st[:, 0:2048])
                chained_act(st[:, 2048:4096], st[:, 2048:4096], AF.Exp,
                            scale=-TEMPERATURE)
                nc.sync.dma_start(out[ts(qt, 128), 2048:4096], st[:, 2048:4096])
            else:
                chained_act(st[:], st[:], AF.Exp, scale=-TEMPERATURE)
                nc.sync.dma_start(out[ts(qt, 128), :], st[:])


def build_nc():
    nc = bacc.Bacc("TRN2", target_bir_lowering=False, debug=False)
    xq = nc.dram_tensor("xq", [NQ, D], F32, kind="ExternalInput").ap()
    xk = nc.dram_tensor("xk", [NK, D], F32, kind="ExternalInput").ap()
    W = nc.dram_tensor("W", [D, D], F32, kind="ExternalInput").ap()
    b = nc.dram_tensor("b", [D, 1], F32, kind="ExternalInput").ap()
    out = nc.dram_tensor("out", [NQ, NK], F32, kind="ExternalOutput").ap()
    with tile.TileContext(nc) as tc:
        with ExitStack() as ctx:
            kernel_body(ctx, tc, out, xq, xk, W, b)
    nc.compile()
    return nc


_NC_CACHE = None


def _get_nc():
    global _NC_CACHE
    if _NC_CACHE is None:
        _NC_CACHE = build_nc()
    return _NC_CACHE


def _run(x, W, b, trace=False, **spmd_kwargs):
    from concourse.bass_utils import run_bass_kernel_spmd

    x = np.asarray(x, dtype=np.float32)
    W = np.asarray(W, dtype=np.float32)
    b = np.asarray(b, dtype=np.float32).reshape(D, 1)
    nc = _get_nc()
    in_maps = []
    for c in range(N_CORES):
        bi, qh = c // 2, c % 2
        in_maps.append({
            "xq": np.ascontiguousarray(x[bi, qh * NQ:(qh + 1) * NQ, :]),
            "xk": np.ascontiguousarray(x[bi]),
            "W": W,
            "b": b,
        })
    res = run_bass_kernel_spmd(
        nc, in_maps, core_ids=list(range(N_CORES)), trace=trace, **spmd_kwargs
    )
    out = np.empty((B, N, N), dtype=np.float32)
    for c in range(N_CORES):
        bi, qh = c // 2, c % 2
        out[bi, qh * NQ:(qh + 1) * NQ, :] = res.results[c]["out"]
    return out, res


def kernel(x, W, b):
    out, _ = _run(x, W, b)
    return out


# revision 58
# speedup vs baseline: 1.0131x; 1.0131x over previous
"""Self-contained TRN2 Bass kernel for nn_EuclideanSimilarity.

Full-input contract: kernel(x, W, b) with
  x [4, 4096, 128] f32, W [128, 128] f32, b [128] f32
returns out [4, 4096, 4096] f32 = exp(-pairwise_euclidean_dist(x @ W.T + b)).

Sharding: 8 cores, core c -> (batch c//2, query-half c%2); each core computes
its [2048, 4096] block of the pairwise-similarity matrix against the full
key set of its batch (SPMD — identical program, different input slices).

Per-core pipeline: transpose x via PE, hT = W @ xT + b (fp32r matmul),
then d2[m,n] = sq[m] + sq[n] - 2 h_m.h_n assembled in PSUM. The PE's
fast fp32r path only carries ~12 mantissa bits, which would destroy the
near-diagonal cancellation, so the gram term uses hi/lo split-precision
(h = hi + lo, each fp32r): gram = g_hi.k_hi + g_hi.k_lo + g_lo.k_hi
(with g = -2h), the key norms enter via an augmented K=2 matmul with
fp32r hi/lo rows, and the query norm is added at full fp32 by the DVE
drain (tensor_scalar: out = max(psum + sq_q[m], 0), which also fuses the
relu while evacuating PSUM). sqrt and exp(-x) run on the scalar engine,
explicitly order-chained in batches so the sqrt/exp activation-table
sets are not thrashed, and each 128-row output tile leaves through one
2 MiB contiguous DMA. PSUM cycles through 4x[128,1024] slots, each
completed by 8 back-to-back matmuls, to keep the PE clock-gate warm.
"""

from contextlib import ExitStack

import numpy as np

import concourse.mybir as mybir
import concourse.tile as tile
from concourse.tile import add_dep_helper
from concourse import bacc
from concourse.bass import ts
from concourse.masks import make_identity

F32 = mybir.dt.float32
F32R = mybir.dt.float32r
AF = mybir.ActivationFunctionType
ALU = mybir.AluOpType

B = 4
N = 4096
NQ = 2048  # query rows per core
NK = 4096  # key rows per core
D = 128
TEMPERATURE = 1.0
NQT = NQ // 128  # query tiles per core
N_CORES = 8


def kernel_body(ctx: ExitStack, tc: tile.TileContext, out, xq, xk, W, b):
    nc = tc.nc

    consts = ctx.enter_context(tc.tile_pool(name="consts", bufs=1))
    # first ACT op is a dummy sqrt: loads the sqrt table set up front, so the
    # setup Identity ops (present in every set) keep it resident and the first
    # real sqrt pays no table load at the setup/main boundary
    scrap = consts.tile([1, 8], F32)
    nc.gpsimd.memset(scrap[:], 1.0)
    nc.scalar.activation(scrap[:], scrap[:], AF.Sqrt)
    ident = consts.tile([128, 128], F32)
    make_identity(nc, ident[:])

    w_sb = consts.tile([128, 128], F32)
    nc.sync.dma_start(w_sb[:], W[:, :])
    b_sb = consts.tile([128, 1], F32)
    nc.sync.dma_start(b_sb[:], b[:, :])
    bm2_sb = consts.tile([128, 1], F32)
    nc.scalar.mul(bm2_sb[:], b_sb[:], -2.0)
    ones_f32 = consts.tile([128, 512], F32)
    nc.gpsimd.memset(ones_f32[:], 1.0)
    ones_col = consts.tile([128, 1], F32)    # lhsT for the f32 sq matmul
    nc.vector.tensor_copy(ones_col[:], ones_f32[:, 0:1])

    # persistent main-loop operands (hi/lo split for fp32-grade gram)
    h_pool = ctx.enter_context(tc.tile_pool(name="h", bufs=1))
    hk_hi = h_pool.tile([128, NK], F32R)
    hk_lo = h_pool.tile([128, NK], F32R)
    gq_hi = h_pool.tile([128, NQ], F32R)   # g = -2*h (queries)
    gq_lo = h_pool.tile([128, NQ], F32R)

    aug_pool = ctx.enter_context(tc.tile_pool(name="aug", bufs=1))
    # d2 += sum_k ones2[k,m] * aug_k[k,n] = sq_k_hi[n] + sq_k_lo[n];
    # sq_q[m] is added per-partition by the DVE relu (full fp32, no split)
    aug_k = aug_pool.tile([2, NK], F32R)   # rows: sq_k_hi, sq_k_lo
    ones2 = aug_pool.tile([2, 128], F32R)  # constant lhsT for the aug matmul
    nc.vector.tensor_copy(ones2[:], ones_f32[0:2, 0:128])
    sqq_cols = aug_pool.tile([128, NQT], F32)  # sq_q in column-per-qtile form

    xk_r = xk.rearrange("(t p) d -> p t d", p=128)
    xq_r = xq.rearrange("(t p) d -> p t d", p=128)

    # ---------------- setup phase (scoped pools) ----------------
    with tc.tile_pool(name="setup_sb", bufs=6) as ssb, \
         tc.tile_pool(name="setup_ps", bufs=2, space="PSUM") as sps, \
         tc.tile_pool(name="rows", bufs=1) as rows_pool:

        wt_ps = sps.tile([128, 512], F32, tag="wt", bufs=1)
        nc.tensor.transpose(wt_ps[:, 0:128], w_sb[:], ident[:])
        wt_sb = consts.tile([128, 128], F32R)
        nc.vector.tensor_copy(wt_sb[:], wt_ps[:, 0:128])

        # single-partition staging row for raw query norms (fp32, 4*|h|^2)
        sqq_row = rows_pool.tile([1, NQ], F32)

        def do_chunks(nchunks, x_r, hi_dst, lo_dst, is_q):
            for c in range(nchunks):
                tagn = "q" if is_q else "k"
                xin = ssb.tile([128, 512], F32, tag="xin", name=f"xin_{tagn}{c}")
                nc.sync.dma_start(
                    xin[:].rearrange("p (t d) -> p t d", d=D),
                    x_r[:, 4 * c:4 * c + 4, :],
                )
                tp = sps.tile([128, 512], F32, tag="tp", bufs=3, name=f"tp_{tagn}{c}")
                for j in range(4):
                    nc.tensor.transpose(
                        tp[:, ts(j, 128)], xin[:, ts(j, 128)], ident[:]
                    )
                xt = ssb.tile([128, 512], F32R, tag="xt", name=f"xt_{tagn}{c}")
                nc.scalar.activation(xt[:], tp[:], AF.Identity)
                hps = sps.tile([128, 512], F32, tag="hps", bufs=2, name=f"hps_{tagn}{c}")
                nc.tensor.matmul(hps[:], wt_sb[:], xt[:], start=True, stop=True)
                hf = ssb.tile([128, 512], F32, tag="hf", name=f"hf_{tagn}{c}")
                if is_q:  # g = -2*(W@xT) - 2b
                    nc.scalar.activation(
                        hf[:], hps[:], AF.Identity, bias=bm2_sb[:, 0:1],
                        scale=-2.0,
                    )
                else:
                    nc.scalar.activation(
                        hf[:], hps[:], AF.Identity, bias=b_sb[:, 0:1]
                    )
                # hi/lo split of h (or g)
                nc.gpsimd.tensor_copy(hi_dst[:, ts(c, 512)], hf[:])
                nc.gpsimd.tensor_tensor(
                    lo_dst[:, ts(c, 512)], hf[:], hi_dst[:, ts(c, 512)],
                    ALU.subtract,
                )
                # squared norms, also hi/lo so the K=128 sum keeps f32 grade
                s2f = ssb.tile([128, 512], F32, tag="s2f", name=f"s2f_{tagn}{c}")
                nc.vector.tensor_mul(s2f[:], hf[:], hf[:])
                sqps = sps.tile([128, 512], F32, tag="sqps", bufs=2, name=f"sqps_{tagn}{c}")
                # plain-f32 matmul (2-pass internally) keeps the norm exact
                nc.tensor.matmul(
                    sqps[0:1, :], ones_col[:], s2f[:], start=True, stop=True
                )
                if is_q:  # raw 4*|h|^2; the 1/4 scale is applied at transpose
                    nc.scalar.activation(
                        sqq_row[0:1, ts(c, 512)], sqps[0:1, :], AF.Identity
                    )
                else:
                    nc.scalar.activation(
                        aug_k[0:1, ts(c, 512)], sqps[0:1, :], AF.Identity
                    )
                    sk = rows_pool.tile([1, 512], F32R, tag="sklo", bufs=2,
                                        name=f"sklo{c}")
                    nc.vector.tensor_tensor(
                        sk[:], sqps[0:1, :], aug_k[0:1, ts(c, 512)],
                        ALU.subtract,
                    )
                    nc.sync.dma_start(aug_k[1:2, ts(c, 512)], sk[:])

        do_chunks(NQ // 512, xq_r, gq_hi, gq_lo, True)
        # transpose sq_q row into column-per-qtile layout via tiny PE transposes
        sqq_ps = sps.tile([128, 512], F32, tag="sqps", bufs=2, name="sqq_ps")
        for qt in range(NQT):
            nc.tensor.transpose(
                sqq_ps[:, qt:qt + 1], sqq_row[0:1, ts(qt, 128)], ident[0:1, 0:1]
            )
        nc.vector.tensor_scalar_mul(sqq_cols[:], sqq_ps[:, 0:NQT], 0.25)
        do_chunks(NK // 512, xk_r, hk_hi, hk_lo, False)

    # ---------------- main loop ----------------
    stage_pool = ctx.enter_context(tc.tile_pool(name="stage", bufs=8))
    d2_ps = ctx.enter_context(tc.tile_pool(name="d2", bufs=8, space="PSUM"))
    NC = NK // 512  # 8 key chunks

    last_act = [None]

    def chained_act(*args, chain=True, **kwargs):
        bi = nc.scalar.activation(*args, **kwargs)
        if chain and last_act[0] is not None:
            # arg order: (waiter, dependency) - this op waits on the previous
            add_dep_helper(bi.ins, last_act[0].ins, sync=False,
                           reason="act-table-order")
        last_act[0] = bi
        return bi

    NH = NK // 1024  # 4 slots of 2 key-chunks each
    spans = [(0, 3), (3, 7), (7, 11), (11, 14), (14, 16)]
    assert spans[-1][1] == NQT
    for g0, g1 in spans:
        group = []
        for qt in range(g0, g1):
            st = stage_pool.tile([128, NK], F32, tag="st", name=f"st{qt}")
            for c in range(NC):
                ps = d2_ps.tile([128, 512], F32, tag="d2", name=f"d2_{qt}_{c}")
                # each slot = one 512-column, completed by 4 back-to-back mms
                nc.tensor.matmul(
                    ps[:], gq_hi[:, ts(qt, 128)], hk_hi[:, ts(c, 512)],
                    start=True, stop=False,
                )
                nc.tensor.matmul(
                    ps[:], gq_hi[:, ts(qt, 128)], hk_lo[:, ts(c, 512)],
                    start=False, stop=False,
                )
                nc.tensor.matmul(
                    ps[:], gq_lo[:, ts(qt, 128)], hk_hi[:, ts(c, 512)],
                    start=False, stop=False,
                )
                nc.tensor.matmul(
                    ps[:], ones2[:], aug_k[:, ts(c, 512)],
                    start=False, stop=True,
                )
                nc.vector.tensor_scalar(
                    st[:, ts(c, 512)], ps[:], sqq_cols[:, qt:qt + 1], 0.0,
                    ALU.add, ALU.max,
                )
            if g0 == 0 and qt < 3:
                # pipeline-fill phase: sqrt per 2048-half starts ~2 slots earlier
                chained_act(st[:, 0:2048], st[:, 0:2048], AF.Sqrt)
                chained_act(st[:, 2048:4096], st[:, 2048:4096], AF.Sqrt)
            else:
                chained_act(st[:], st[:], AF.Sqrt)
            group.append((qt, st))
        for qt, st in group:
            if qt == NQT - 1:
                # final tile: halve exp+DMA so the last DMA overlaps the exp
                chained_act(st[:, 0:2048], st[:, 0:2048], AF.Exp,
                            scale=-TEMPERATURE)
                nc.sync.dma_start(out[ts(qt, 128), 0:2048], st[:, 0:2048])
                chained_act(st[:, 2048:4096], st[:, 2048:4096], AF.Exp,
                            scale=-TEMPERATURE)
                nc.sync.dma_start(out[ts(qt, 128), 2048:4096], st[:, 2048:4096])
            else:
                chained_act(st[:], st[:], AF.Exp, scale=-TEMPERATURE)
                nc.sync.dma_start(out[ts(qt, 128), :], st[:])


def build_nc():
    nc = bacc.Bacc("TRN2", target_bir_lowering=False, debug=False)
    xq = nc.dram_tensor("xq", [NQ, D], F32, kind="ExternalInput").ap()
    xk = nc.dram_tensor("xk", [NK, D], F32, kind="ExternalInput").ap()
    W = nc.dram_tensor("W", [D, D], F32, kind="ExternalInput").ap()
    b = nc.dram_tensor("b", [D, 1], F32, kind="ExternalInput").ap()
    out = nc.dram_tensor("out", [NQ, NK], F32, kind="ExternalOutput").ap()
    with tile.TileContext(nc) as tc:
        with ExitStack() as ctx:
            kernel_body(ctx, tc, out, xq, xk, W, b)
    nc.compile()
    return nc


_NC_CACHE = None


def _get_nc():
    global _NC_CACHE
    if _NC_CACHE is None:
        _NC_CACHE = build_nc()
    return _NC_CACHE


def _run(x, W, b, trace=False, **spmd_kwargs):
    from concourse.bass_utils import run_bass_kernel_spmd

    x = np.asarray(x, dtype=np.float32)
    W = np.asarray(W, dtype=np.float32)
    b = np.asarray(b, dtype=np.float32).reshape(D, 1)
    nc = _get_nc()
    in_maps = []
    for c in range(N_CORES):
        bi, qh = c // 2, c % 2
        in_maps.append({
            "xq": np.ascontiguousarray(x[bi, qh * NQ:(qh + 1) * NQ, :]),
            "xk": np.ascontiguousarray(x[bi]),
            "W": W,
            "b": b,
        })
    res = run_bass_kernel_spmd(
        nc, in_maps, core_ids=list(range(N_CORES)), trace=trace, **spmd_kwargs
    )
    out = np.empty((B, N, N), dtype=np.float32)
    for c in range(N_CORES):
        bi, qh = c // 2, c % 2
        out[bi, qh * NQ:(qh + 1) * NQ, :] = res.results[c]["out"]
    return out, res


def kernel(x, W, b):
    out, _ = _run(x, W, b)
    return out
